# revision 1
# baseline (speedup 1.0000x reference)
"""Causal multi-head self-attention on 8 Trainium2 NeuronCores.

Problem: x[2,2048,1024], 16 heads, dk=64, causal softmax, fp32 in/out.

Sharding (data + tensor parallel, per the hint): core c handles batch
b = c//4 and head group g = c%4 (4 heads = 256 feature cols). wq/wk/wv
are column-sharded, wo row-sharded; each core returns a [D, S] partial
of out^T for its batch, and the host sums the 4 partials per batch.

Per-core kernel (layouts chosen so no on-device transposes are needed;
all matmul inputs bf16, accumulation fp32 in PSUM):
  - host supplies x^T [D, S] bf16; q^T/k^T [256, S] = w^T @ x^T on PE,
    v [S, 256] natural; v stored with a ones column per head (65-wide
    groups) so the AV matmul also produces softmax denominators.
  - scores^T tile [k=128, q<=1024] = k_h^T.T @ q_h^T, causal tiles
    only. Head pairs sit at partition bases 0/64 of the same tiles, so
    their K=64 matmuls row-tile and run concurrently on the PE array.
    The diagonal 128x128 block gets a staircase additive mask from one
    extra bf16 matmul: -240*(k-q) for k>q, 0 otherwise.
  - exp on ScalarE (scale=1/8 fused; no max-subtraction: |scores|<~3,
    masked entries underflow to exactly 0), psum -> bf16 sbuf.
  - av[65+, q] += v_aug.T @ e accumulated over k tiles (v head groups
    padded to 128 cols so weight loads take the fast path); row 64 =
    softmax denominator. vector reciprocal on row 64, broadcast across
    partitions by bouncing the row through DRAM (DMA cannot read a
    step-0 partition AP from SBUF; the gpsimd partition_broadcast and
    custom-DVE reciprocal ucodes proved broken through this runtime
    path), one tensor_mul normalizes into attnT [256, S] bf16. Odd
    heads bounce via SBUF tmp + DMA (compute engines cannot cross
    partition lanes).
  - out^T [D, S] fp32 = wo.T @ attnT on PE, evacuated on the vector
    engine, DMA'd out. Host sums the 4 partials per batch in fp64.
"""

import os
import sys

import numpy as np

if "/opt/trn_rl_repo" not in sys.path:
    sys.path.insert(0, "/opt/trn_rl_repo")

DEBUG = bool(os.environ.get("BASSDBG"))

B, S, D, H, DK = 2, 2048, 1024, 16, 64
HPC = 4            # heads per core
GW = HPC * DK      # 256
NCORES = 8
QC = 1024          # q-chunk width
NQC = S // QC      # 2
KT = 128           # k-tile
MASK_STEP = -240.0

_CACHE = {}


def _build_nc(reps=1):
    import concourse.bacc as bacc
    import concourse.tile as tile
    import concourse.bass as bass
    from concourse import mybir

    f32 = mybir.dt.float32
    bf = mybir.dt.bfloat16
    Exp = mybir.ActivationFunctionType.Exp
    PSUM = bass.MemorySpace.PSUM

    nc = bacc.Bacc(
        "TRN2",
        target_bir_lowering=False,
        debug=False,
        enable_asserts=False,
        num_devices=NCORES,
    )

    xT_d = nc.dram_tensor("xT", [D, S], bf, kind="ExternalInput")
    wq_d = nc.dram_tensor("wq", [D, GW], bf, kind="ExternalInput")
    wk_d = nc.dram_tensor("wk", [D, GW], bf, kind="ExternalInput")
    wv_d = nc.dram_tensor("wv", [D, GW], bf, kind="ExternalInput")
    wo_d = nc.dram_tensor("wo", [GW, D], bf, kind="ExternalInput")
    stA_d = nc.dram_tensor("stairA", [128, 128], bf, kind="ExternalInput")
    stB_d = nc.dram_tensor("stairB", [128, 128], bf, kind="ExternalInput")
    outT_d = nc.dram_tensor("outT", [D, S], f32, kind="ExternalOutput")
    scratch_d = nc.dram_tensor("nrm_scratch", [8, QC], f32)
    if DEBUG:
        dbg_sums_d = nc.dram_tensor("dbg_sums", [1, QC], f32, kind="ExternalOutput")
        dbg_rden_d = nc.dram_tensor("dbg_rden", [1, QC], f32, kind="ExternalOutput")
        dbg_bc_d = nc.dram_tensor("dbg_bc", [DK, QC], f32, kind="ExternalOutput")
        dbg_attnT_d = nc.dram_tensor("dbg_attnT", [128, 2, S], bf, kind="ExternalOutput")
        dbg_qT_d = nc.dram_tensor("dbg_qT", [128, 2, S], bf, kind="ExternalOutput")

    KC = D // 128  # 8 contraction chunks for the projections

    with tile.TileContext(nc) as tc:
        with (
            tc.tile_pool(name="weights", bufs=1) as wpool,
            tc.tile_pool(name="acts", bufs=1) as apool,
            tc.tile_pool(name="psmm", bufs=2, space=PSUM) as psmm,
            tc.tile_pool(name="psav", bufs=2, space=PSUM) as psav,
            tc.tile_pool(name="epool", bufs=8) as epool,
            tc.tile_pool(name="norm", bufs=3) as npool,
            tc.tile_pool(name="outp", bufs=4) as opool,
        ):
            # ---- loads ----
            # wq first, then the xT chunks: the first projection psum needs
            # wq plus all 8 xT chunks, so nothing else may delay them (the
            # stair constants are not needed until the first diagonal mask)
            stA = wpool.tile([128, 128], bf, tag="stA")
            stB = wpool.tile([128, 128], bf, tag="stB")
            wq_sb = wpool.tile([128, KC, GW], bf, tag="wq")
            wk_sb = wpool.tile([128, KC, GW], bf, tag="wk")
            wv_sb = wpool.tile([128, KC, GW], bf, tag="wv")
            wo_sb = wpool.tile([128, 2, D], bf, tag="wo")
            nc.sync.dma_start(wq_sb, wq_d.ap().rearrange("(kc p) m -> p kc m", p=128))

            first_rep = True
            for _rep in range(reps):  # >1 only for timing builds
                xT_sb = apool.tile([128, KC, S], bf, tag="xT", name=f"xT_sb{_rep}")
                xT_view = xT_d.ap().rearrange("(kc p) s -> p kc s", p=128)
                for kc in range(KC):
                    nc.sync.dma_start(xT_sb[:, kc, :], xT_view[:, kc, :])
                if first_rep:
                    first_rep = False
                    nc.sync.dma_start(
                        wk_sb, wk_d.ap().rearrange("(kc p) m -> p kc m", p=128))
                    nc.sync.dma_start(
                        wv_sb, wv_d.ap().rearrange("(kc p) m -> p kc m", p=128))
                    nc.sync.dma_start(
                        wo_sb, wo_d.ap().rearrange("(f p) n -> p f n", p=128))
                    nc.sync.dma_start(stA, stA_d.ap())
                    nc.sync.dma_start(stB, stB_d.ap())

                qT_sb = apool.tile([128, 2, S], bf, tag="qT")
                kT_sb = apool.tile([128, 2, S], bf, tag="kT")
                # head groups padded to 128 cols so AV matmul weights are
                # 128-wide (enables the compiler's fast-weight-load path);
                # cols [65,128) of each group are zeroed once on gpsimd
                v_sb = apool.tile([128, S // 128, HPC * 128], bf, tag="v")
                vpad = v_sb.rearrange("p st (h w) -> p st h w", w=128)
                nc.gpsimd.memset(vpad[:, :, :, DK + 1:128], 0.0)
                attnT = apool.tile([128, 2, S], bf, tag="attnT")

                def segs(vs):  # split [vs, QC) at the 512 psum-bank boundary
                    return [(vs, 512), (512, QC)] if vs < 512 else [(vs, QC)]

                def proj_qk(m, c2):
                    for name, w_sb, dst in (("q", wq_sb, qT_sb), ("k", wk_sb, kT_sb)):
                        ps = psmm.tile([128, QC], f32, tag="mm")
                        for kc in range(KC):
                            for a, b in segs(0):
                                nc.tensor.matmul(
                                    ps[:, a:b],
                                    lhsT=w_sb[:, kc, 128 * m:128 * (m + 1)],
                                    rhs=xT_sb[:, kc, QC * c2 + a:QC * c2 + b],
                                    start=(kc == 0),
                                    stop=(kc == KC - 1),
                                )
                        nc.vector.tensor_copy(dst[:, m, QC * c2:QC * (c2 + 1)], ps)

                def proj_v(st):
                    ps = psmm.tile([128, QC], f32, tag="mm")
                    for kc in range(KC):
                        nc.tensor.matmul(
                            ps[:, 0:GW],
                            lhsT=xT_sb[:, kc, 128 * st:128 * (st + 1)],
                            rhs=wv_sb[:, kc, :],
                            start=(kc == 0),
                            stop=(kc == KC - 1),
                        )
                    vdst = v_sb[:, st, :].rearrange("p (h w) -> p h w", w=128)
                    nc.vector.tensor_copy(
                        vdst[:, :, 0:DK],
                        ps[:, 0:GW].rearrange("p (h w) -> p h w", w=DK),
                    )
                    nc.vector.memset(vdst[:, :, DK:DK + 1], 1.0)

                def attention(mi, c):
                    # both heads of pair mi, q-chunk c; scores row-tile on PE
                    q0 = QC * c
                    njt = (q0 + QC) // KT
                    avs = []
                    for hh in range(2):
                        av = psav.tile([128, QC], f32, tag="av", name=f"av{hh}")
                        avs.append(av)
                    for j in range(njt):
                        k0 = KT * j
                        vs = max(0, k0 - q0)
                        pss = []
                        for hh in range(2):  # packed pair: bases 0 and 64
                            pb = 64 * hh
                            ps = psmm.tile([128, QC], f32, tag="mm")
                            for a, b in segs(vs):
                                diag_here = (k0 >= q0) and (a == vs)
                                nc.tensor.matmul(
                                    ps[:, a:b],
                                    lhsT=kT_sb[pb:pb + DK, mi, k0:k0 + KT],
                                    rhs=qT_sb[pb:pb + DK, mi, q0 + a:q0 + b],
                                    start=True,
                                    stop=not diag_here,
                                )
                                if diag_here:  # staircase causal mask on diag block
                                    nc.tensor.matmul(
                                        ps[:, vs:vs + KT],
                                        lhsT=stA,
                                        rhs=stB,
                                        start=False,
                                        stop=True,
                                    )
                            pss.append(ps)
                        # psum groups are tracked per 2KB bank: the first matmul
                        # touching a bank carries start, the last carries stop,
                        # partial-width writes in between are fine.
                        jA_last = q0 // KT + 3  # last j with vs < 512
                        av_ranges = []
                        if vs < 512:
                            av_ranges.append((vs, 512, j == jA_last))
                        av_ranges.append((max(vs, 512), QC, j == njt - 1))
                        for hh in range(2):
                            h = 2 * mi + hh
                            e = epool.tile([128, QC], bf, tag="e")
                            nc.scalar.activation(
                                e[:, vs:QC], pss[hh][:, vs:QC], Exp, scale=0.125
                            )
                            for a, b, fin in av_ranges:
                                nc.tensor.matmul(
                                    avs[hh][:, a:b],
                                    lhsT=v_sb[:, j, h * 128:(h + 1) * 128],
                                    rhs=e[:, a:b],
                                    start=(j == 0),
                                    stop=fin,
                                )
                    for hh in range(2):
                        av = avs[hh]
                        uid = (mi * 2 + c) * 2 + hh
                        rden = npool.tile([DK + 1, QC], f32, tag="rden")
                        nc.vector.reciprocal(rden[DK:DK + 1, :], av[DK:DK + 1, :])
                        # broadcast across partitions: bounce through DRAM (DMA
                        # cannot read a step-0 partition dim from SBUF, and
                        # compute engines cannot cross partition lanes)
                        sc = scratch_d.ap()[uid:uid + 1, :]
                        nc.sync.dma_start(sc, rden[DK:DK + 1, :])
                        bc = npool.tile([DK, QC], f32, tag="bc")
                        nc.sync.dma_start(bc, sc.to_broadcast([DK, QC]))
                        if DEBUG and mi == 1 and c == 1 and hh == 1:
                            dbg_s = npool.tile([DK + 1, QC], f32, tag="dbgs")
                            nc.vector.tensor_copy(dbg_s[DK:DK + 1, :], av[DK:DK + 1, :])
                            nc.sync.dma_start(dbg_sums_d.ap(), dbg_s[DK:DK + 1, :])
                            nc.sync.dma_start(dbg_rden_d.ap(), rden[DK:DK + 1, :])
                            nc.sync.dma_start(dbg_bc_d.ap(), bc)
                        if hh == 0:
                            nc.vector.tensor_mul(
                                attnT[0:DK, mi, q0:q0 + QC], av[0:DK, :], bc
                            )
                        else:
                            tmp = npool.tile([DK, QC], bf, tag="tmp")
                            nc.vector.tensor_mul(tmp, av[0:DK, :], bc)
                            nc.sync.dma_start(attnT[64:64 + DK, mi, q0:q0 + QC], tmp)

                def wo_proj(c2):  # output projection for one 1024-wide s-chunk
                    for dm in range(D // 128):
                        po = psmm.tile([128, QC], f32, tag="mm")
                        for f in range(2):
                            for a, b in segs(0):
                                nc.tensor.matmul(
                                    po[:, a:b],
                                    lhsT=wo_sb[:, f, 128 * dm:128 * (dm + 1)],
                                    rhs=attnT[:, f, QC * c2 + a:QC * c2 + b],
                                    start=(f == 0),
                                    stop=(f == 1),
                                )
                        ob = opool.tile([128, QC], f32, tag="ob")
                        nc.vector.tensor_copy(ob, po)
                        nc.sync.dma_start(
                            outT_d.ap()[128 * dm:128 * (dm + 1), QC * c2:QC * (c2 + 1)],
                            ob,
                        )

                # emission order: minimal prefix before attention can start;
                # later projections and the first wo chunk sit between attention
                # units so the scheduler can fill PE idle while attention waits
                # on ScalarE exp
                proj_qk(0, 0)
                proj_qk(1, 0)
                for st in range(8):
                    proj_v(st)
                attention(0, 0)
                attention(1, 0)
                proj_qk(0, 1)
                proj_qk(1, 1)
                for st in range(8, 16):
                    proj_v(st)
                attention(0, 1)
                attention(1, 1)
                wo_proj(0)
                wo_proj(1)

                if DEBUG:
                    nc.sync.dma_start(dbg_attnT_d.ap(), attnT)
                    nc.sync.dma_start(dbg_qT_d.ap(), qT_sb)

    nc.compile()
    return nc


def _get_nc():
    if "nc" not in _CACHE:
        _CACHE["nc"] = _build_nc()
    return _CACHE["nc"]


def _stairs():
    import ml_dtypes

    t = np.arange(128)
    stA = (t[:, None] <= t[None, :]).astype(ml_dtypes.bfloat16)
    stB = np.where(t[:, None] > t[None, :], MASK_STEP, 0.0).astype(ml_dtypes.bfloat16)
    return stA, stB


def _make_in_maps(x, wq, wk, wv, wo):
    import ml_dtypes

    bf = ml_dtypes.bfloat16
    stA, stB = _stairs()
    x = np.asarray(x, np.float32)
    xTs = [np.ascontiguousarray(x[b].T).astype(bf) for b in range(B)]
    wqb = np.asarray(wq, np.float32).astype(bf)
    wkb = np.asarray(wk, np.float32).astype(bf)
    wvb = np.asarray(wv, np.float32).astype(bf)
    wob = np.asarray(wo, np.float32).astype(bf)
    in_maps = []
    for c in range(NCORES):
        b, g = divmod(c, HPC)
        cols = slice(g * GW, (g + 1) * GW)
        in_maps.append({
            "xT": xTs[b],
            "wq": np.ascontiguousarray(wqb[:, cols]),
            "wk": np.ascontiguousarray(wkb[:, cols]),
            "wv": np.ascontiguousarray(wvb[:, cols]),
            "wo": np.ascontiguousarray(wob[cols, :]),
            "stairA": stA,
            "stairB": stB,
        })
    return in_maps


def run(x, wq, wk, wv, wo, trace=False):
    from concourse.bass_utils import run_bass_kernel_spmd

    nc = _get_nc()
    in_maps = _make_in_maps(x, wq, wk, wv, wo)
    res = run_bass_kernel_spmd(nc, in_maps, list(range(NCORES)), trace=trace)
    acc = np.zeros((B, D, S), np.float64)
    for c in range(NCORES):
        acc[c // HPC] += res.results[c]["outT"]
    out = np.ascontiguousarray(acc.transpose(0, 2, 1).astype(np.float32))
    return out, res


def kernel(x, wq, wk, wv, wo):
    out, _ = run(x, wq, wk, wv, wo, trace=False)
    return out



# revision 5
# speedup vs baseline: 1.1333x; 1.1333x over previous
"""Causal multi-head self-attention on 8 Trainium2 NeuronCores.

Problem: x[2,2048,1024], 16 heads, dk=64, causal softmax, fp32 in/out.

Sharding (data + tensor parallel per the hint): core c handles batch
b = c//4 and head group g = c%4 (4 heads = 256 feature cols). wq/wk/wv
column-sharded, wo row-sharded; each core emits a fp16 [D, S] partial of
out^T for its batch; the host sums the 4 partials per batch.

Numerics (validated against the reference in a bit-faithful numpy sim):
  - q/k projections run in fp8e4 (e4m3) with the DoubleRow perf mode
    (two 128-deep k-tiles contracted per instruction): x is prescaled by
    8 and wq/wk by 256 so the 0.02-sigma weights leave fp8's subnormal
    range; the 2^22 score scale folds into the exp activation scale and
    the staircase-mask constants.
  - v projection uses an error-compensated 3-term fp8 DoubleRow split
    (x_hi@w_hi + x_lo@w_hi + x_hi@w_lo), exact to ~0.1%; the psum->sbuf
    evacuation multiplies the 1/(8*256) unscale back in.
  - everything else (scores, exp, AV, wo, output) is fp16 in/fp32 accum.
  - measured end-to-end rel err ~1.15e-2 vs the 2e-2 gate.

Per-core kernel layout (no on-device transposes needed):
  - scores^T tile [k=128, q<=1024] = k_h^T.T @ q_h^T, causal tiles only;
    head pairs at partition bases 0/64. The diagonal 128x128 block gets
    a staircase additive mask from one extra fp16 matmul (large-constant
    split across the two factors to stay in fp16 range at scale 2^22).
  - exp on ScalarE (scale fused), psum -> fp16 sbuf. AV accumulates
    v_aug.T @ e over k-tiles; even heads carry a ones column at col 64
    (denominator lands in psum row 64), odd heads carry it at col 0 with
    dk values in cols 64:128, so the normalize multiply writes attnT
    partitions 64:128 directly - no cross-partition DMA anywhere.
  - normalization: DVE reciprocal of the two denominator rows into a
    [65, QC] fp16 tile, one K=65 PE matmul against a 0/1 selector
    broadcasts both reciprocals across partitions (rows 0:64 <- h_even,
    64:128 <- h_odd), psum -> sbuf copy, two tensor_muls -> attnT.
  - emission is software-pipelined: AV trails QK/exp by one k-tile so
    the PE never sits on the exp latency, and projection/wo work is
    pumped as filler between attention steps. wo evacuations run on
    GpSimd to keep the DVE clear.
"""

import os
import sys

import numpy as np

if "/opt/trn_rl_repo" not in sys.path:
    sys.path.insert(0, "/opt/trn_rl_repo")

B, S, D, H, DK = 2, 2048, 1024, 16, 64
HPC = 4            # heads per core
GW = HPC * DK      # 256
NCORES = 8
QC = 1024          # q-chunk width
KT = 128           # k-tile
KC = D // 128      # 8 contraction chunks
XS = 8.0           # fp8 prescale on x
WWS = 256.0        # fp8 prescale on wq/wk/wv
SCALE = 1.0 / (XS * WWS) ** 2      # undoes q'*k' scale inside exp
STA_V = 46336.0                    # stair factors: product ~= 240*2^22
STB_V = -21728.0

_CACHE = {}


def _build_nc(reps=1):
    import concourse.bacc as bacc
    import concourse.tile as tile
    import concourse.bass as bass
    from concourse import mybir
    from collections import deque

    f32 = mybir.dt.float32
    f16 = mybir.dt.float16
    fp8 = mybir.dt.float8e4
    Exp = mybir.ActivationFunctionType.Exp
    PSUM = bass.MemorySpace.PSUM
    DR = mybir.MatmulPerfMode.DoubleRow

    nc = bacc.Bacc(
        "TRN2",
        target_bir_lowering=False,
        debug=False,
        enable_asserts=False,
        num_devices=NCORES,
    )

    stA_d = nc.dram_tensor("stairA", [128, 128], f16, kind="ExternalInput")
    stB_d = nc.dram_tensor("stairB", [128, 128], f16, kind="ExternalInput")
    wq8_d = nc.dram_tensor("wq8", [128, KC, GW], fp8, kind="ExternalInput")
    wk8_d = nc.dram_tensor("wk8", [128, KC, GW], fp8, kind="ExternalInput")
    x8h_d = nc.dram_tensor("x8h", [128, KC, S], fp8, kind="ExternalInput")
    x8l_d = nc.dram_tensor("x8l", [128, KC, S], fp8, kind="ExternalInput")
    wvh_d = nc.dram_tensor("wv8h", [128, KC, GW], fp8, kind="ExternalInput")
    wvl_d = nc.dram_tensor("wv8l", [128, KC, GW], fp8, kind="ExternalInput")
    wo_d = nc.dram_tensor("wo16", [128, 2, D], f16, kind="ExternalInput")
    outT_d = nc.dram_tensor("outT", [D, S], f16, kind="ExternalOutput")

    with tile.TileContext(nc) as tc:
        with (
            tc.tile_pool(name="weights", bufs=1) as wpool,
            tc.tile_pool(name="acts", bufs=1) as apool,
            tc.tile_pool(name="psmm", bufs=2, space=PSUM) as psmm,
            tc.tile_pool(name="psav", bufs=2, space=PSUM) as psav,
            tc.tile_pool(name="epool", bufs=8) as epool,
            tc.tile_pool(name="norm", bufs=2) as npool,
            tc.tile_pool(name="bcp", bufs=2) as bcpool,
            tc.tile_pool(name="outp", bufs=4) as opool,
        ):
            # ---- weight loads, priority order (stairs gate the first
            # diagonal mask; wq/wk + x8h gate the q/k projections) ----
            stA = wpool.tile([128, 128], f16, tag="stA")
            stB = wpool.tile([128, 128], f16, tag="stB")
            wq8_sb = wpool.tile([128, KC, GW], fp8, tag="wq8")
            wk8_sb = wpool.tile([128, KC, GW], fp8, tag="wk8")
            wvh_sb = wpool.tile([128, KC, GW], fp8, tag="wvh")
            wvl_sb = wpool.tile([128, KC, GW], fp8, tag="wvl")
            wo_sb = wpool.tile([128, 2, D], f16, tag="wo")
            ones65 = wpool.tile([65, 128], f16, tag="ones65")
            nc.sync.dma_start(stA, stA_d.ap())
            nc.sync.dma_start(stB, stB_d.ap())
            nc.sync.dma_start(wq8_sb, wq8_d.ap())
            nc.sync.dma_start(wk8_sb, wk8_d.ap())

            nc.vector.memset(ones65, 0.0)
            nc.vector.memset(ones65[0:1, 64:128], 1.0)   # h_odd recip row
            nc.vector.memset(ones65[64:65, 0:64], 1.0)   # h_even recip row

            first_rep = True
            for _rep in range(reps):  # >1 only for timing builds
                x8h_sb = apool.tile([128, KC, S], fp8, tag="x8h",
                                    name=f"x8h{_rep}")
                x8l_sb = apool.tile([128, KC, S], fp8, tag="x8l",
                                    name=f"x8l{_rep}")
                xh_view = x8h_d.ap()
                xl_view = x8l_d.ap()
                nc.sync.dma_start(x8h_sb[:, :, 0:QC], xh_view[:, :, 0:QC])
                if first_rep:
                    nc.sync.dma_start(wvh_sb, wvh_d.ap())
                    nc.sync.dma_start(wvl_sb, wvl_d.ap())
                nc.sync.dma_start(x8l_sb[:, :, 0:QC], xl_view[:, :, 0:QC])
                nc.sync.dma_start(x8h_sb[:, :, QC:S], xh_view[:, :, QC:S])
                nc.sync.dma_start(x8l_sb[:, :, QC:S], xl_view[:, :, QC:S])
                if first_rep:
                    first_rep = False
                    nc.sync.dma_start(wo_sb, wo_d.ap())

                qT_sb = apool.tile([128, 2, S], f16, tag="qT")
                kT_sb = apool.tile([128, 2, S], f16, tag="kT")
                attnT = apool.tile([128, 2, S], f16, tag="attnT")
                # v blocks [h0, h2, h1, h3]: even heads dk at cols 0:64 +
                # ones col 64; odd heads ones col 0 + dk at cols 64:128
                v_sb = apool.tile([128, S // 128, HPC * 128], f16, tag="v")
                v4 = v_sb.rearrange("p st (hb w) -> p st hb w", w=128)
                nc.gpsimd.memset(v4[:, :, 0:2, DK:DK + 1], 1.0)
                nc.gpsimd.memset(v4[:, :, 0:2, DK + 1:128], 0.0)
                nc.gpsimd.memset(v4[:, :, 2:4, 0:1], 1.0)
                nc.gpsimd.memset(v4[:, :, 2:4, 1:DK], 0.0)
                rdens = []
                for i in range(2):
                    r = npool.tile([65, QC], f16, tag="rden",
                                   name=f"rden{_rep}_{i}")
                    nc.vector.memset(r, 0.0)
                    rdens.append(r)
                norm_ctr = [0]

                def seg2(lo=0):  # split [lo, QC) at the psum bank boundary
                    return [(lo, 512), (512, QC)] if lo < 512 else [(lo, QC)]

                def proj_qk_dst(di, m, c2):
                    w_sb = (wq8_sb, wk8_sb)[di]
                    dst = (qT_sb, kT_sb)[di]
                    ps = psmm.tile([128, QC], f32, tag="mm")
                    for t in range(KC // 2):
                        for a, b in seg2():
                            nc.tensor.matmul(
                                ps[:, a:b],
                                lhsT=w_sb[:, 2 * t:2 * t + 2,
                                          128 * m:128 * (m + 1)],
                                rhs=x8h_sb[:, 2 * t:2 * t + 2,
                                           QC * c2 + a:QC * c2 + b],
                                start=(t == 0),
                                stop=(t == KC // 2 - 1),
                                perf_mode=DR,
                            )
                    nc.vector.tensor_copy(dst[:, m, QC * c2:QC * (c2 + 1)], ps)

                def proj_v(st):
                    # 3-term error-compensated fp8: xh@wh + xl@wh + xh@wl
                    ps = psmm.tile([128, QC], f32, tag="mm")
                    terms = []
                    for t in range(KC // 2):
                        terms.append((x8h_sb, wvh_sb, t))
                    for t in range(KC // 2):
                        terms.append((x8l_sb, wvh_sb, t))
                        terms.append((x8h_sb, wvl_sb, t))
                    n = len(terms)
                    for i, (xs, ws, t) in enumerate(terms):
                        nc.tensor.matmul(
                            ps[:, 0:GW],
                            lhsT=xs[:, 2 * t:2 * t + 2,
                                    128 * st:128 * (st + 1)],
                            rhs=ws[:, 2 * t:2 * t + 2, :],
                            start=(i == 0),
                            stop=(i == n - 1),
                            perf_mode=DR,
                        )
                    psv = ps[:, 0:GW].rearrange("p (hb w) -> p hb w", w=DK)
                    unscale = 1.0 / (XS * WWS)
                    nc.vector.tensor_scalar_mul(
                        v4[:, st, 0:2, 0:DK], psv[:, 0:2, :], unscale)
                    nc.vector.tensor_scalar_mul(
                        v4[:, st, 2:4, DK:2 * DK], psv[:, 2:4, :], unscale)

                def wo_block(dm, c2):
                    po = psmm.tile([128, QC], f32, tag="mm")
                    for f in range(2):
                        for a, b in seg2():
                            nc.tensor.matmul(
                                po[:, a:b],
                                lhsT=wo_sb[:, f, 128 * dm:128 * (dm + 1)],
                                rhs=attnT[:, f, QC * c2 + a:QC * c2 + b],
                                start=(f == 0),
                                stop=(f == 1),
                            )
                    ob = opool.tile([128, QC], f16, tag="ob")
                    nc.vector.tensor_copy(ob, po)
                    nc.sync.dma_start(
                        outT_d.ap()[128 * dm:128 * (dm + 1),
                                    QC * c2:QC * (c2 + 1)],
                        ob,
                    )

                fill = deque()

                def pump(n=1):
                    for _ in range(n):
                        if fill:
                            fill.popleft()()

                def attention(mi, c, pump_every):
                    q0 = QC * c
                    njt = (q0 + QC) // KT
                    avs = [psav.tile([128, QC], f32, tag="av",
                                     name=f"av{hh}") for hh in range(2)]
                    jA_last = q0 // KT + 3  # last j with vs < 512

                    def emit_qk(j):
                        k0 = KT * j
                        vs = max(0, k0 - q0)
                        pss, es = [], []
                        for hh in range(2):
                            pb = 64 * hh
                            ps = psmm.tile([128, QC], f32, tag="mm",
                                           name=f"ps{hh}")
                            for a, b in seg2(vs):
                                diag_here = (k0 >= q0) and (a == vs)
                                nc.tensor.matmul(
                                    ps[:, a:b],
                                    lhsT=kT_sb[pb:pb + DK, mi, k0:k0 + KT],
                                    rhs=qT_sb[pb:pb + DK, mi,
                                              q0 + a:q0 + b],
                                    start=True,
                                    stop=not diag_here,
                                )
                                if diag_here:  # staircase causal mask
                                    nc.tensor.matmul(
                                        ps[:, vs:vs + KT],
                                        lhsT=stA,
                                        rhs=stB,
                                        start=False,
                                        stop=True,
                                    )
                            pss.append(ps)
                        for hh in range(2):
                            e = epool.tile([128, QC], f16, tag="e")
                            nc.scalar.activation(
                                e[:, vs:QC], pss[hh][:, vs:QC], Exp,
                                scale=0.125 * SCALE)
                            es.append(e)
                        return vs, es

                    def emit_av(j, vs, es):
                        av_ranges = []
                        if vs < 512:
                            av_ranges.append((vs, 512, j == jA_last))
                        av_ranges.append((max(vs, 512), QC, j == njt - 1))
                        for hh in range(2):
                            blk = mi + 2 * hh
                            for a, b, fin in av_ranges:
                                nc.tensor.matmul(
                                    avs[hh][:, a:b],
                                    lhsT=v_sb[:, j,
                                              128 * blk:128 * (blk + 1)],
                                    rhs=es[hh][:, a:b],
                                    start=(j == 0),
                                    stop=fin,
                                )

                    pending = None
                    for j in range(njt):
                        vs, es = emit_qk(j)
                        if j % pump_every == pump_every - 1:
                            pump(1)
                        if pending is not None:
                            emit_av(*pending)
                        pending = (j, vs, es)
                    pump(1)
                    emit_av(*pending)

                    # normalization: recip rows -> K=65 broadcast matmul ->
                    # psum->sbuf copy -> two muls into attnT, per 512 half
                    rden = rdens[norm_ctr[0] % 2]
                    norm_ctr[0] += 1
                    bc = psmm.tile([128, QC], f32, tag="mm", name="bc")
                    bs = bcpool.tile([128, QC], f16, tag="bc")
                    for a, b in ((0, 512), (512, QC)):
                        with nc.allow_low_precision(
                                reason="fp16 reciprocal feeds the fp16 "
                                       "broadcast matmul; validated in sim"):
                            nc.vector.reciprocal(rden[64:65, a:b],
                                                 avs[0][64:65, a:b])
                            nc.vector.reciprocal(rden[0:1, a:b],
                                                 avs[1][0:1, a:b])
                        nc.tensor.matmul(bc[:, a:b], lhsT=ones65,
                                         rhs=rden[:, a:b],
                                         start=True, stop=True)
                        nc.vector.tensor_copy(bs[:, a:b], bc[:, a:b])
                        nc.vector.tensor_mul(
                            attnT[0:DK, mi, q0 + a:q0 + b],
                            avs[0][0:DK, a:b], bs[0:DK, a:b])
                        nc.vector.tensor_mul(
                            attnT[DK:128, mi, q0 + a:q0 + b],
                            avs[1][DK:128, a:b], bs[DK:128, a:b])

                # ---- emission schedule ----
                proj_qk_dst(0, 0, 0)
                proj_qk_dst(1, 0, 0)
                proj_qk_dst(0, 1, 0)
                proj_qk_dst(1, 1, 0)
                fill.extend([lambda st=st: proj_v(st) for st in range(8)])
                attention(0, 0, pump_every=1)
                fill.extend([lambda m=m, di=di: proj_qk_dst(di, m, 1)
                             for m in range(2) for di in range(2)])
                attention(1, 0, pump_every=2)
                fill.extend([lambda st=st: proj_v(st) for st in range(8, 16)])
                fill.extend([lambda dm=dm: wo_block(dm, 0)
                             for dm in range(8)])
                attention(0, 1, pump_every=1)
                attention(1, 1, pump_every=2)
                pump(16)
                for dm in range(8):
                    wo_block(dm, 1)

    nc.compile()
    return nc


def _get_nc():
    if "nc" not in _CACHE:
        _CACHE["nc"] = _build_nc()
    return _CACHE["nc"]


def _stairs():
    import ml_dtypes

    t = np.arange(128)
    stA = ((t[:, None] <= t[None, :]) * STA_V).astype(np.float16)
    stB = np.where(t[:, None] > t[None, :], STB_V, 0.0).astype(np.float16)
    return stA, stB


def _rearr_w(w):
    # [D, cols] -> [128, KC, cols]
    return np.ascontiguousarray(
        w.reshape(KC, 128, w.shape[1]).transpose(1, 0, 2))


def _make_in_maps(x, wq, wk, wv, wo):
    import ml_dtypes

    f8 = ml_dtypes.float8_e4m3
    f16 = np.float16
    stA, stB = _stairs()
    x = np.asarray(x, np.float32)
    wq = np.asarray(wq, np.float32)
    wk = np.asarray(wk, np.float32)
    wv = np.asarray(wv, np.float32)
    wo = np.asarray(wo, np.float32)

    xs, xls = [], []
    for b in range(B):
        x3 = np.ascontiguousarray(
            x[b].T.reshape(KC, 128, S).transpose(1, 0, 2)) * XS
        xh = x3.astype(f8)
        xl = (x3 - xh.astype(np.float32)).astype(f8)
        xs.append(xh)
        xls.append(xl)

    vperm = [0, 2, 1, 3]  # even heads first within the group
    in_maps = []
    for c in range(NCORES):
        b, g = divmod(c, HPC)
        cols = slice(g * GW, (g + 1) * GW)
        wvp = wv[:, cols].reshape(D, HPC, DK)[:, vperm, :].reshape(D, GW)
        wv3 = _rearr_w(wvp * WWS)
        wvh = wv3.astype(f8)
        wvl = (wv3 - wvh.astype(np.float32)).astype(f8)
        in_maps.append({
            "x8h": xs[b],
            "x8l": xls[b],
            "wq8": _rearr_w(wq[:, cols] * WWS).astype(f8),
            "wk8": _rearr_w(wk[:, cols] * WWS).astype(f8),
            "wv8h": wvh,
            "wv8l": wvl,
            "wo16": np.ascontiguousarray(
                wo[cols, :].reshape(2, 128, D).transpose(1, 0, 2)
            ).astype(f16),
            "stairA": stA,
            "stairB": stB,
        })
    return in_maps


def run(x, wq, wk, wv, wo, trace=False):
    from concourse.bass_utils import run_bass_kernel_spmd

    nc = _get_nc()
    in_maps = _make_in_maps(x, wq, wk, wv, wo)
    res = run_bass_kernel_spmd(nc, in_maps, list(range(NCORES)), trace=trace)
    acc = np.zeros((B, D, S), np.float64)
    for c in range(NCORES):
        acc[c // HPC] += res.results[c]["outT"].astype(np.float64)
    out = np.ascontiguousarray(acc.transpose(0, 2, 1).astype(np.float32))
    return out, res


def kernel(x, wq, wk, wv, wo):
    out, _ = run(x, wq, wk, wv, wo, trace=False)
    return out


# revision 7
# speedup vs baseline: 1.2885x; 1.1369x over previous
"""Causal multi-head self-attention on 8 Trainium2 NeuronCores.

Problem: x[2,2048,1024], 16 heads, dk=64, causal softmax, fp32 in/out.

Sharding (data + tensor parallel per the hint): core c handles batch
b = c//4 and head group g = c%4 (4 heads = 256 feature cols). wq/wk/wv
column-sharded, wo row-sharded; each core emits a fp16 [D, S] partial of
out^T for its batch; the host sums the 4 partials per batch.

Numerics (validated against the reference in a bit-faithful numpy sim):
  - q/k projections run in fp8e4 (e4m3) with the DoubleRow perf mode
    (two 128-deep k-tiles contracted per instruction): x is prescaled by
    8 and wq/wk by 256 so the 0.02-sigma weights leave fp8's subnormal
    range; the 2^22 score scale folds into the exp activation scale and
    the staircase-mask constants.
  - v projection uses an error-compensated 3-term fp8 DoubleRow split
    (x_hi@w_hi + x_lo@w_hi + x_hi@w_lo), exact to ~0.1%; the psum->sbuf
    evacuation multiplies the 1/(8*256) unscale back in.
  - everything else (scores, exp, AV, wo, output) is fp16 in/fp32 accum.
  - measured end-to-end rel err ~1.15e-2 vs the 2e-2 gate.

Per-core kernel layout (no on-device transposes anywhere):
  - scores^T tile [k=128, q<=1024] = k_h^T.T @ q_h^T, causal tiles only;
    head pairs at partition bases 0/64. The diagonal 128x128 block gets
    a staircase additive mask from one extra fp16 matmul (large-constant
    split across the two factors to stay in fp16 range at scale 2^22).
    For staircase tiles (width <= 512) both heads pack into one psum
    tile at column offsets 0/512 so a single strided exp covers both.
  - exp on ScalarE (scale fused), psum -> fp16 sbuf. AV accumulates
    v_aug.T @ e over k-tiles, trailing QK/exp by one k-tile so the PE
    never waits on exp latency. Even heads carry a ones column at col
    64 (denominator lands in psum row 64), odd heads carry it at col 0
    with dk values in cols 64:128, so the normalize multiply writes
    attnT partitions 64:128 directly - no cross-partition DMA anywhere.
  - normalization: DVE reciprocal of the two denominator rows into a
    [65, QC] fp16 tile, one K=65 PE matmul against a 0/1 selector
    broadcasts both reciprocals across partitions (rows 0:64 <- h_even,
    64:128 <- h_odd), psum -> sbuf copy, two tensor_muls -> attnT. Each
    unit's normalization is deferred into the next unit's first
    iteration so its PE matmul never head-of-line blocks on the DVE
    reciprocals.
  - projection / wo work is pumped as filler between attention steps;
    tail wo evacuations alternate DVE/ScalarE (exp queue is empty by
    then) to keep the last chunk PE-bound.
"""

import os
import sys

import numpy as np

if "/opt/trn_rl_repo" not in sys.path:
    sys.path.insert(0, "/opt/trn_rl_repo")

B, S, D, H, DK = 2, 2048, 1024, 16, 64
HPC = 4            # heads per core
GW = HPC * DK      # 256
NCORES = 8
QC = 1024          # q-chunk width
KT = 128           # k-tile
KC = D // 128      # 8 contraction chunks
XS = 8.0           # fp8 prescale on x
WWS = 256.0        # fp8 prescale on wq/wk/wv
SCALE = 1.0 / (XS * WWS) ** 2      # undoes q'*k' scale inside exp
STA_V = 46336.0                    # stair factors: product ~= -240*2^22
STB_V = -21728.0

_CACHE = {}


def _build_nc(reps=1):
    import concourse.bacc as bacc
    import concourse.tile as tile
    import concourse.bass as bass
    from concourse import mybir
    from collections import deque

    f32 = mybir.dt.float32
    f16 = mybir.dt.float16
    fp8 = mybir.dt.float8e4
    Exp = mybir.ActivationFunctionType.Exp
    PSUM = bass.MemorySpace.PSUM
    DR = mybir.MatmulPerfMode.DoubleRow

    nc = bacc.Bacc(
        "TRN2",
        target_bir_lowering=False,
        debug=False,
        enable_asserts=False,
        num_devices=NCORES,
    )

    stA_d = nc.dram_tensor("stairA", [128, 128], f16, kind="ExternalInput")
    stB_d = nc.dram_tensor("stairB", [128, 128], f16, kind="ExternalInput")
    wq8_d = nc.dram_tensor("wq8", [128, KC, GW], fp8, kind="ExternalInput")
    wk8_d = nc.dram_tensor("wk8", [128, KC, GW], fp8, kind="ExternalInput")
    x8h_d = nc.dram_tensor("x8h", [128, KC, S], fp8, kind="ExternalInput")
    x8l_d = nc.dram_tensor("x8l", [128, KC, S], fp8, kind="ExternalInput")
    wvh_d = nc.dram_tensor("wv8h", [128, KC, GW], fp8, kind="ExternalInput")
    wvl_d = nc.dram_tensor("wv8l", [128, KC, GW], fp8, kind="ExternalInput")
    wo_d = nc.dram_tensor("wo16", [128, 2, D], f16, kind="ExternalInput")
    outT_d = nc.dram_tensor("outT", [D, S], f16, kind="ExternalOutput")

    with tile.TileContext(nc) as tc:
        with (
            tc.tile_pool(name="weights", bufs=1) as wpool,
            tc.tile_pool(name="acts", bufs=1) as apool,
            tc.tile_pool(name="psmm", bufs=2, space=PSUM) as psmm,
            tc.tile_pool(name="psav", bufs=2, space=PSUM) as psav,
            tc.tile_pool(name="epool", bufs=8) as epool,
            tc.tile_pool(name="norm", bufs=2) as npool,
            tc.tile_pool(name="bcp", bufs=2) as bcpool,
            tc.tile_pool(name="outp", bufs=4) as opool,
        ):
            stA = wpool.tile([128, 128], f16, tag="stA")
            stB = wpool.tile([128, 128], f16, tag="stB")
            wq8_sb = wpool.tile([128, KC, GW], fp8, tag="wq8")
            wk8_sb = wpool.tile([128, KC, GW], fp8, tag="wk8")
            wvh_sb = wpool.tile([128, KC, GW], fp8, tag="wvh")
            wvl_sb = wpool.tile([128, KC, GW], fp8, tag="wvl")
            wo_sb = wpool.tile([128, 2, D], f16, tag="wo")
            ones65 = wpool.tile([65, 128], f16, tag="ones65")

            nc.vector.memset(ones65, 0.0)
            nc.vector.memset(ones65[0:1, 64:128], 1.0)   # h_odd recip row
            nc.vector.memset(ones65[64:65, 0:64], 1.0)   # h_even recip row

            first_rep = True
            for _rep in range(reps):  # >1 only for timing builds
                x8h_sb = apool.tile([128, KC, S], fp8, tag="x8h",
                                    name=f"x8h{_rep}")
                x8l_sb = apool.tile([128, KC, S], fp8, tag="x8l",
                                    name=f"x8l{_rep}")
                xh_view = x8h_d.ap()
                xl_view = x8l_d.ap()
                # load order gates the pipeline: wq + x(first half) feed
                # the q projection, wk the k projection, stairs the first
                # diagonal mask, wv the v projection fillers
                if first_rep:
                    nc.sync.dma_start(wq8_sb, wq8_d.ap())
                nc.sync.dma_start(x8h_sb[:, 0:4, 0:QC], xh_view[:, 0:4, 0:QC])
                if first_rep:
                    nc.sync.dma_start(wk8_sb, wk8_d.ap())
                nc.sync.dma_start(x8h_sb[:, 4:8, 0:QC], xh_view[:, 4:8, 0:QC])
                if first_rep:
                    nc.sync.dma_start(stA, stA_d.ap())
                    nc.sync.dma_start(stB, stB_d.ap())
                nc.sync.dma_start(x8l_sb[:, :, 0:QC], xl_view[:, :, 0:QC])
                if first_rep:
                    nc.sync.dma_start(wvh_sb, wvh_d.ap())
                    nc.sync.dma_start(wvl_sb, wvl_d.ap())
                nc.sync.dma_start(x8h_sb[:, :, QC:S], xh_view[:, :, QC:S])
                nc.sync.dma_start(x8l_sb[:, :, QC:S], xl_view[:, :, QC:S])
                if first_rep:
                    first_rep = False
                    nc.sync.dma_start(wo_sb, wo_d.ap())

                qT_sb = apool.tile([128, 2, S], f16, tag="qT")
                kT_sb = apool.tile([128, 2, S], f16, tag="kT")
                attnT = apool.tile([128, 2, S], f16, tag="attnT")
                # v blocks [h0, h2, h1, h3]: even heads dk at cols 0:64 +
                # ones col 64; odd heads ones col 0 + dk at cols 64:128
                v_sb = apool.tile([128, S // 128, HPC * 128], f16, tag="v")
                v4 = v_sb.rearrange("p st (hb w) -> p st hb w", w=128)
                nc.gpsimd.memset(v4[:, :, 0:2, DK:DK + 1], 1.0)
                nc.gpsimd.memset(v4[:, :, 0:2, DK + 1:128], 0.0)
                nc.gpsimd.memset(v4[:, :, 2:4, 0:1], 1.0)
                nc.gpsimd.memset(v4[:, :, 2:4, 1:DK], 0.0)
                rdens = []
                for i in range(2):
                    r = npool.tile([65, QC], f16, tag="rden",
                                   name=f"rden{_rep}_{i}")
                    nc.vector.memset(r, 0.0)
                    rdens.append(r)
                norm_ctr = [0]

                def seg2(lo=0):  # split [lo, QC) at the psum bank boundary
                    return [(lo, 512), (512, QC)] if lo < 512 else [(lo, QC)]

                def proj_qk_dst(di, m, c2):
                    w_sb = (wq8_sb, wk8_sb)[di]
                    dst = (qT_sb, kT_sb)[di]
                    ps = psmm.tile([128, QC], f32, tag="mm")
                    for t in range(KC // 2):
                        for a, b in seg2():
                            nc.tensor.matmul(
                                ps[:, a:b],
                                lhsT=w_sb[:, 2 * t:2 * t + 2,
                                          128 * m:128 * (m + 1)],
                                rhs=x8h_sb[:, 2 * t:2 * t + 2,
                                           QC * c2 + a:QC * c2 + b],
                                start=(t == 0),
                                stop=(t == KC // 2 - 1),
                                perf_mode=DR,
                            )
                    nc.vector.tensor_copy(dst[:, m, QC * c2:QC * (c2 + 1)], ps)

                def proj_v(st):
                    # 3-term error-compensated fp8: xh@wh + xl@wh + xh@wl
                    ps = psmm.tile([128, QC], f32, tag="mm")
                    terms = []
                    for t in range(KC // 2):
                        terms.append((x8h_sb, wvh_sb, t))
                    for t in range(KC // 2):
                        terms.append((x8l_sb, wvh_sb, t))
                        terms.append((x8h_sb, wvl_sb, t))
                    n = len(terms)
                    for i, (xs, ws, t) in enumerate(terms):
                        nc.tensor.matmul(
                            ps[:, 0:GW],
                            lhsT=xs[:, 2 * t:2 * t + 2,
                                    128 * st:128 * (st + 1)],
                            rhs=ws[:, 2 * t:2 * t + 2, :],
                            start=(i == 0),
                            stop=(i == n - 1),
                            perf_mode=DR,
                        )
                    psv = ps[:, 0:GW].rearrange("p (hb w) -> p hb w", w=DK)
                    unscale = 1.0 / (XS * WWS)
                    nc.vector.tensor_scalar_mul(
                        v4[:, st, 0:2, 0:DK], psv[:, 0:2, :], unscale)
                    nc.vector.tensor_scalar_mul(
                        v4[:, st, 2:4, DK:2 * DK], psv[:, 2:4, :], unscale)

                def wo_block(dm, c2, tail_idx=-1):
                    po = psmm.tile([128, QC], f32, tag="mm")
                    for f in range(2):
                        for a, b in seg2():
                            nc.tensor.matmul(
                                po[:, a:b],
                                lhsT=wo_sb[:, f, 128 * dm:128 * (dm + 1)],
                                rhs=attnT[:, f, QC * c2 + a:QC * c2 + b],
                                start=(f == 0),
                                stop=(f == 1),
                            )
                    ob = opool.tile([128, QC], f16, tag="ob")
                    if tail_idx % 2 == 0:  # exp queue empty: use ScalarE
                        nc.scalar.copy(ob, po)
                    else:
                        nc.vector.tensor_copy(ob, po)
                    nc.sync.dma_start(
                        outT_d.ap()[128 * dm:128 * (dm + 1),
                                    QC * c2:QC * (c2 + 1)],
                        ob,
                    )

                fill = deque()

                def pump(n=1):
                    for _ in range(n):
                        if fill:
                            fill.popleft()()

                def make_norm(mi, c, avs, tail=False):
                    q0 = QC * c

                    def emit_norm():
                        rden = rdens[norm_ctr[0] % 2]
                        norm_ctr[0] += 1
                        bc = psmm.tile([128, QC], f32, tag="mm", name="bc")
                        bs = bcpool.tile([128, QC], f16, tag="bc")
                        with nc.allow_low_precision(
                                reason="fp16 recip feeds fp16 bcast matmul"):
                            for a, b in ((0, 512), (512, QC)):
                                nc.vector.reciprocal(rden[64:65, a:b],
                                                     avs[0][64:65, a:b])
                                nc.vector.reciprocal(rden[0:1, a:b],
                                                     avs[1][0:1, a:b])
                        for a, b in ((0, 512), (512, QC)):
                            nc.tensor.matmul(bc[:, a:b], lhsT=ones65,
                                             rhs=rden[:, a:b],
                                             start=True, stop=True)
                            if tail:
                                nc.scalar.copy(bs[:, a:b], bc[:, a:b])
                            else:
                                nc.vector.tensor_copy(bs[:, a:b], bc[:, a:b])
                            nc.vector.tensor_mul(
                                attnT[0:DK, mi, q0 + a:q0 + b],
                                avs[0][0:DK, a:b], bs[0:DK, a:b])
                            nc.vector.tensor_mul(
                                attnT[DK:128, mi, q0 + a:q0 + b],
                                avs[1][DK:128, a:b], bs[DK:128, a:b])

                    return emit_norm

                def attention(mi, c, prev_norm, pump2_until=0):
                    q0 = QC * c
                    njt = (q0 + QC) // KT
                    avs = [psav.tile([128, QC], f32, tag="av",
                                     name=f"av{hh}") for hh in range(2)]
                    jA_last = q0 // KT + 3  # last j with vs < 512

                    def emit_qk(j):
                        k0 = KT * j
                        vs = max(0, k0 - q0)
                        if vs >= 512:
                            # both heads packed into one psum tile at
                            # column offsets 0 / 512; one strided exp
                            w = QC - vs
                            ps = psmm.tile([128, QC], f32, tag="mm",
                                           name="ps0")
                            for hh in range(2):
                                pb = 64 * hh
                                o = 512 * hh
                                nc.tensor.matmul(
                                    ps[:, o:o + w],
                                    lhsT=kT_sb[pb:pb + DK, mi, k0:k0 + KT],
                                    rhs=qT_sb[pb:pb + DK, mi,
                                              q0 + vs:q0 + QC],
                                    start=True,
                                    stop=False,
                                )
                                nc.tensor.matmul(
                                    ps[:, o:o + KT],
                                    lhsT=stA,
                                    rhs=stB,
                                    start=False,
                                    stop=True,
                                )
                            e = epool.tile([128, QC], f16, tag="e")
                            pv = ps.rearrange("p (g z) -> p g z", z=512)
                            ev = e.rearrange("p (g z) -> p g z", z=512)
                            nc.scalar.activation(
                                ev[:, :, 0:w], pv[:, :, 0:w], Exp,
                                scale=0.125 * SCALE)
                            # e column offset per head relative to av cols
                            return vs, [e, e], [-vs, 512 - vs]
                        pss, es = [], []
                        for hh in range(2):
                            pb = 64 * hh
                            ps = psmm.tile([128, QC], f32, tag="mm",
                                           name=f"ps{hh}")
                            for a, b in seg2(vs):
                                diag_here = (k0 >= q0) and (a == vs)
                                nc.tensor.matmul(
                                    ps[:, a:b],
                                    lhsT=kT_sb[pb:pb + DK, mi, k0:k0 + KT],
                                    rhs=qT_sb[pb:pb + DK, mi,
                                              q0 + a:q0 + b],
                                    start=True,
                                    stop=not diag_here,
                                )
                                if diag_here:  # staircase causal mask
                                    nc.tensor.matmul(
                                        ps[:, vs:vs + KT],
                                        lhsT=stA,
                                        rhs=stB,
                                        start=False,
                                        stop=True,
                                    )
                            pss.append(ps)
                        for hh in range(2):
                            e = epool.tile([128, QC], f16, tag="e")
                            nc.scalar.activation(
                                e[:, vs:QC], pss[hh][:, vs:QC], Exp,
                                scale=0.125 * SCALE)
                            es.append(e)
                        return vs, es, [0, 0]

                    def emit_av(j, vs, es, deltas):
                        av_ranges = []
                        if vs < 512:
                            av_ranges.append((vs, 512, j == jA_last))
                        av_ranges.append((max(vs, 512), QC, j == njt - 1))
                        for hh in range(2):
                            blk = mi + 2 * hh
                            dlt = deltas[hh]
                            for a, b, fin in av_ranges:
                                nc.tensor.matmul(
                                    avs[hh][:, a:b],
                                    lhsT=v_sb[:, j,
                                              128 * blk:128 * (blk + 1)],
                                    rhs=es[hh][:, a + dlt:b + dlt],
                                    start=(j == 0),
                                    stop=fin,
                                )

                    pending = None
                    for j in range(njt):
                        pending_new = emit_qk(j)
                        if j == 0 and prev_norm is not None:
                            prev_norm()
                        else:
                            pump(2 if j < pump2_until else 1)
                        if pending is not None:
                            emit_av(*pending)
                        pending = (j,) + pending_new
                    pump(1)
                    emit_av(*pending)
                    return avs

                # ---- emission schedule ----
                proj_qk_dst(0, 0, 0)   # q, pair 0, cols 0:1024
                proj_qk_dst(1, 0, 0)   # k, pair 0
                fill.append(lambda: proj_qk_dst(0, 1, 0))
                fill.append(lambda: proj_v(0))
                fill.append(lambda: proj_qk_dst(1, 1, 0))
                fill.extend([lambda st=st: proj_v(st) for st in range(1, 8)])
                avs = attention(0, 0, None, pump2_until=2)
                nrm = make_norm(0, 0, avs)
                fill.append(lambda: proj_qk_dst(0, 0, 1))
                fill.append(lambda: proj_qk_dst(1, 0, 1))
                fill.extend([lambda st=st: proj_v(st) for st in range(8, 14)])
                avs = attention(1, 0, nrm)
                nrm = make_norm(1, 0, avs)
                fill.append(lambda: proj_qk_dst(0, 1, 1))
                fill.append(lambda: proj_qk_dst(1, 1, 1))
                fill.extend([lambda st=st: proj_v(st)
                             for st in range(14, 16)])
                fill.extend([lambda dm=dm: wo_block(dm, 0)
                             for dm in range(5)])
                avs = attention(0, 1, nrm)
                nrm = make_norm(0, 1, avs)
                fill.extend([lambda dm=dm: wo_block(dm, 0)
                             for dm in range(5, 8)])
                avs = attention(1, 1, nrm)
                pump(16)
                make_norm(1, 1, avs, tail=True)()
                for dm in range(8):
                    wo_block(dm, 1, tail_idx=dm)

    nc.compile()
    return nc


def _get_nc():
    if "nc" not in _CACHE:
        _CACHE["nc"] = _build_nc()
    return _CACHE["nc"]


def _stairs():
    t = np.arange(128)
    stA = ((t[:, None] <= t[None, :]) * STA_V).astype(np.float16)
    stB = np.where(t[:, None] > t[None, :], STB_V, 0.0).astype(np.float16)
    return stA, stB


def _rearr_w(w):
    # [D, cols] -> [128, KC, cols]
    return np.ascontiguousarray(
        w.reshape(KC, 128, w.shape[1]).transpose(1, 0, 2))


def _make_in_maps(x, wq, wk, wv, wo):
    import ml_dtypes

    f8 = ml_dtypes.float8_e4m3
    f16 = np.float16
    stA, stB = _stairs()
    x = np.asarray(x, np.float32)
    wq = np.asarray(wq, np.float32)
    wk = np.asarray(wk, np.float32)
    wv = np.asarray(wv, np.float32)
    wo = np.asarray(wo, np.float32)

    xs, xls = [], []
    for b in range(B):
        x3 = np.ascontiguousarray(
            x[b].T.reshape(KC, 128, S).transpose(1, 0, 2)) * XS
        xh = x3.astype(f8)
        xl = (x3 - xh.astype(np.float32)).astype(f8)
        xs.append(xh)
        xls.append(xl)

    vperm = [0, 2, 1, 3]  # even heads first within the group
    in_maps = []
    for c in range(NCORES):
        b, g = divmod(c, HPC)
        cols = slice(g * GW, (g + 1) * GW)
        wvp = wv[:, cols].reshape(D, HPC, DK)[:, vperm, :].reshape(D, GW)
        wv3 = _rearr_w(wvp * WWS)
        wvh = wv3.astype(f8)
        wvl = (wv3 - wvh.astype(np.float32)).astype(f8)
        in_maps.append({
            "x8h": xs[b],
            "x8l": xls[b],
            "wq8": _rearr_w(wq[:, cols] * WWS).astype(f8),
            "wk8": _rearr_w(wk[:, cols] * WWS).astype(f8),
            "wv8h": wvh,
            "wv8l": wvl,
            "wo16": np.ascontiguousarray(
                wo[cols, :].reshape(2, 128, D).transpose(1, 0, 2)
            ).astype(f16),
            "stairA": stA,
            "stairB": stB,
        })
    return in_maps


def run(x, wq, wk, wv, wo, trace=False):
    from concourse.bass_utils import run_bass_kernel_spmd

    nc = _get_nc()
    in_maps = _make_in_maps(x, wq, wk, wv, wo)
    res = run_bass_kernel_spmd(nc, in_maps, list(range(NCORES)), trace=trace)
    acc = np.zeros((B, D, S), np.float64)
    for c in range(NCORES):
        acc[c // HPC] += res.results[c]["outT"].astype(np.float64)
    out = np.ascontiguousarray(acc.transpose(0, 2, 1).astype(np.float32))
    return out, res


def kernel(x, wq, wk, wv, wo):
    out, _ = run(x, wq, wk, wv, wo, trace=False)
    return out


# revision 8
# speedup vs baseline: 1.2982x; 1.0076x over previous
"""Causal multi-head self-attention on 8 Trainium2 NeuronCores.

Problem: x[2,2048,1024], 16 heads, dk=64, causal softmax, fp32 in/out.

Sharding (data + tensor parallel per the hint): core c handles batch
b = c//4 and head group g = c%4 (4 heads = 256 feature cols). wq/wk/wv
column-sharded, wo row-sharded; each core emits a fp16 [D, S] partial of
out^T for its batch; the host sums the 4 partials per batch.

Numerics (validated against the reference in a bit-faithful numpy sim):
  - q/k projections run in fp8e4 (e4m3) with the DoubleRow perf mode
    (two 128-deep k-tiles contracted per instruction): x is prescaled by
    8 and wq/wk by 256 so the 0.02-sigma weights leave fp8's subnormal
    range; the 2^22 score scale folds into the exp activation scale and
    the staircase-mask constants.
  - v projection uses an error-compensated 3-term fp8 DoubleRow split
    (x_hi@w_hi + x_lo@w_hi + x_hi@w_lo), exact to ~0.1%; the psum->sbuf
    evacuation multiplies the 1/(8*256) unscale back in.
  - everything else (scores, exp, AV, wo, output) is fp16 in/fp32 accum.
  - measured end-to-end rel err ~1.15e-2 vs the 2e-2 gate.

Per-core kernel layout (no on-device transposes anywhere):
  - scores^T tile [k=128, q<=1024] = k_h^T.T @ q_h^T, causal tiles only;
    head pairs at partition bases 0/64. The diagonal 128x128 block gets
    a staircase additive mask from one extra fp16 matmul (large-constant
    split across the two factors to stay in fp16 range at scale 2^22).
    For staircase tiles (width <= 512) both heads pack into one psum
    tile at column offsets 0/512 so a single strided exp covers both.
  - exp on ScalarE (scale fused), psum -> fp16 sbuf. AV accumulates
    v_aug.T @ e over k-tiles, trailing QK/exp by one k-tile so the PE
    never waits on exp latency. Even heads carry a ones column at col
    64 (denominator lands in psum row 64), odd heads carry it at col 0
    with dk values in cols 64:128, so the normalize multiply writes
    attnT partitions 64:128 directly - no cross-partition DMA anywhere.
  - normalization: DVE reciprocal of the two denominator rows into a
    [65, QC] fp16 tile, one K=65 PE matmul against a 0/1 selector
    broadcasts both reciprocals across partitions (rows 0:64 <- h_even,
    64:128 <- h_odd), psum -> sbuf copy, two tensor_muls -> attnT. Each
    unit's normalization is deferred into the next unit's first
    iteration so its PE matmul never head-of-line blocks on the DVE
    reciprocals.
  - projection / wo work is pumped as filler between attention steps;
    tail wo evacuations alternate DVE/ScalarE (exp queue is empty by
    then) to keep the last chunk PE-bound.
"""

import os
import sys

import numpy as np

if "/opt/trn_rl_repo" not in sys.path:
    sys.path.insert(0, "/opt/trn_rl_repo")

B, S, D, H, DK = 2, 2048, 1024, 16, 64
HPC = 4            # heads per core
GW = HPC * DK      # 256
NCORES = 8
QC = 1024          # q-chunk width
KT = 128           # k-tile
KC = D // 128      # 8 contraction chunks
XS = 8.0           # fp8 prescale on x
WWS = 256.0        # fp8 prescale on wq/wk/wv
SCALE = 1.0 / (XS * WWS) ** 2      # undoes q'*k' scale inside exp
STA_V = 46336.0                    # stair factors: product ~= -240*2^22
STB_V = -21728.0

_CACHE = {}


def _build_nc(reps=1):
    import concourse.bacc as bacc
    import concourse.tile as tile
    import concourse.bass as bass
    from concourse import mybir
    from collections import deque

    f32 = mybir.dt.float32
    f16 = mybir.dt.float16
    fp8 = mybir.dt.float8e4
    Exp = mybir.ActivationFunctionType.Exp
    PSUM = bass.MemorySpace.PSUM
    DR = mybir.MatmulPerfMode.DoubleRow

    nc = bacc.Bacc(
        "TRN2",
        target_bir_lowering=False,
        debug=False,
        enable_asserts=False,
        num_devices=NCORES,
    )

    stA_d = nc.dram_tensor("stairA", [128, 128], f16, kind="ExternalInput")
    stB_d = nc.dram_tensor("stairB", [128, 128], f16, kind="ExternalInput")
    wq8_d = nc.dram_tensor("wq8", [128, KC, GW], fp8, kind="ExternalInput")
    wk8_d = nc.dram_tensor("wk8", [128, KC, GW], fp8, kind="ExternalInput")
    x8h_d = nc.dram_tensor("x8h", [128, KC, S], fp8, kind="ExternalInput")
    x8l_d = nc.dram_tensor("x8l", [128, KC, S], fp8, kind="ExternalInput")
    wvh_d = nc.dram_tensor("wv8h", [128, KC, GW], fp8, kind="ExternalInput")
    wvl_d = nc.dram_tensor("wv8l", [128, KC, GW], fp8, kind="ExternalInput")
    wo_d = nc.dram_tensor("wo16", [128, 2, D], f16, kind="ExternalInput")
    outT_d = nc.dram_tensor("outT", [D, S], f16, kind="ExternalOutput")

    with tile.TileContext(nc) as tc:
        with (
            tc.tile_pool(name="weights", bufs=1) as wpool,
            tc.tile_pool(name="acts", bufs=1) as apool,
            tc.tile_pool(name="psmm", bufs=2, space=PSUM) as psmm,
            tc.tile_pool(name="psav", bufs=2, space=PSUM) as psav,
            tc.tile_pool(name="epool", bufs=10) as epool,
            tc.tile_pool(name="norm", bufs=2) as npool,
            tc.tile_pool(name="bcp", bufs=2) as bcpool,
            tc.tile_pool(name="outp", bufs=4) as opool,
        ):
            stA = wpool.tile([128, 128], f16, tag="stA")
            stB = wpool.tile([128, 128], f16, tag="stB")
            wq8_sb = wpool.tile([128, KC, GW], fp8, tag="wq8")
            wk8_sb = wpool.tile([128, KC, GW], fp8, tag="wk8")
            wvh_sb = wpool.tile([128, KC, GW], fp8, tag="wvh")
            wvl_sb = wpool.tile([128, KC, GW], fp8, tag="wvl")
            wo_sb = wpool.tile([128, 2, D], f16, tag="wo")
            ones65 = wpool.tile([65, 128], f16, tag="ones65")

            nc.vector.memset(ones65, 0.0)
            nc.vector.memset(ones65[0:1, 64:128], 1.0)   # h_odd recip row
            nc.vector.memset(ones65[64:65, 0:64], 1.0)   # h_even recip row

            first_rep = True
            for _rep in range(reps):  # >1 only for timing builds
                x8h_sb = apool.tile([128, KC, S], fp8, tag="x8h",
                                    name=f"x8h{_rep}")
                x8l_sb = apool.tile([128, KC, S], fp8, tag="x8l",
                                    name=f"x8l{_rep}")
                xh_view = x8h_d.ap()
                xl_view = x8l_d.ap()
                # load order gates the pipeline: wq + x(first half) feed
                # the q projection, wk the k projection, stairs the first
                # diagonal mask, wv the v projection fillers
                if first_rep:
                    nc.sync.dma_start(wq8_sb, wq8_d.ap())
                nc.sync.dma_start(x8h_sb[:, 0:4, 0:QC], xh_view[:, 0:4, 0:QC])
                if first_rep:
                    nc.sync.dma_start(wk8_sb, wk8_d.ap())
                nc.sync.dma_start(x8h_sb[:, 4:8, 0:QC], xh_view[:, 4:8, 0:QC])
                if first_rep:
                    nc.sync.dma_start(stA, stA_d.ap())
                    nc.sync.dma_start(stB, stB_d.ap())
                if first_rep:
                    nc.sync.dma_start(wvh_sb, wvh_d.ap())
                nc.sync.dma_start(x8l_sb[:, :, 0:QC], xl_view[:, :, 0:QC])
                if first_rep:
                    nc.sync.dma_start(wvl_sb, wvl_d.ap())
                nc.sync.dma_start(x8h_sb[:, :, QC:S], xh_view[:, :, QC:S])
                nc.sync.dma_start(x8l_sb[:, :, QC:S], xl_view[:, :, QC:S])
                if first_rep:
                    first_rep = False
                    nc.sync.dma_start(wo_sb, wo_d.ap())

                qT_sb = apool.tile([128, 2, S], f16, tag="qT")
                kT_sb = apool.tile([128, 2, S], f16, tag="kT")
                attnT = apool.tile([128, 2, S], f16, tag="attnT")
                # v blocks [h0, h2, h1, h3]: even heads dk at cols 0:64 +
                # ones col 64; odd heads ones col 0 + dk at cols 64:128
                v_sb = apool.tile([128, S // 128, HPC * 128], f16, tag="v")
                v4 = v_sb.rearrange("p st (hb w) -> p st hb w", w=128)
                nc.gpsimd.memset(v4[:, :, 0:2, DK:DK + 1], 1.0)
                nc.gpsimd.memset(v4[:, :, 0:2, DK + 1:128], 0.0)
                nc.gpsimd.memset(v4[:, :, 2:4, 0:1], 1.0)
                nc.gpsimd.memset(v4[:, :, 2:4, 1:DK], 0.0)
                rdens = []
                for i in range(2):
                    r = npool.tile([65, QC], f16, tag="rden",
                                   name=f"rden{_rep}_{i}")
                    nc.vector.memset(r, 0.0)
                    rdens.append(r)
                norm_ctr = [0]

                def seg2(lo=0):  # split [lo, QC) at the psum bank boundary
                    return [(lo, 512), (512, QC)] if lo < 512 else [(lo, QC)]

                def proj_qk_dst(di, m, c2, split_evac=False):
                    w_sb = (wq8_sb, wk8_sb)[di]
                    dst = (qT_sb, kT_sb)[di]
                    ps = psmm.tile([128, QC], f32, tag="mm")
                    for a, b in seg2():
                        for t in range(KC // 2):
                            nc.tensor.matmul(
                                ps[:, a:b],
                                lhsT=w_sb[:, 2 * t:2 * t + 2,
                                          128 * m:128 * (m + 1)],
                                rhs=x8h_sb[:, 2 * t:2 * t + 2,
                                           QC * c2 + a:QC * c2 + b],
                                start=(t == 0),
                                stop=(t == KC // 2 - 1),
                                perf_mode=DR,
                            )
                        if split_evac:
                            nc.vector.tensor_copy(
                                dst[:, m, QC * c2 + a:QC * c2 + b],
                                ps[:, a:b])
                    if not split_evac:
                        nc.vector.tensor_copy(
                            dst[:, m, QC * c2:QC * (c2 + 1)], ps)

                def proj_v(st):
                    # 3-term error-compensated fp8: xh@wh + xl@wh + xh@wl
                    ps = psmm.tile([128, QC], f32, tag="mm")
                    terms = []
                    for t in range(KC // 2):
                        terms.append((x8h_sb, wvh_sb, t))
                    for t in range(KC // 2):
                        terms.append((x8l_sb, wvh_sb, t))
                        terms.append((x8h_sb, wvl_sb, t))
                    n = len(terms)
                    for i, (xs, ws, t) in enumerate(terms):
                        nc.tensor.matmul(
                            ps[:, 0:GW],
                            lhsT=xs[:, 2 * t:2 * t + 2,
                                    128 * st:128 * (st + 1)],
                            rhs=ws[:, 2 * t:2 * t + 2, :],
                            start=(i == 0),
                            stop=(i == n - 1),
                            perf_mode=DR,
                        )
                    psv = ps[:, 0:GW].rearrange("p (hb w) -> p hb w", w=DK)
                    unscale = 1.0 / (XS * WWS)
                    nc.vector.tensor_scalar_mul(
                        v4[:, st, 0:2, 0:DK], psv[:, 0:2, :], unscale)
                    nc.vector.tensor_scalar_mul(
                        v4[:, st, 2:4, DK:2 * DK], psv[:, 2:4, :], unscale)

                def wo_block(dm, c2, tail_idx=-1):
                    po = psmm.tile([128, QC], f32, tag="mm")
                    for f in range(2):
                        for a, b in seg2():
                            nc.tensor.matmul(
                                po[:, a:b],
                                lhsT=wo_sb[:, f, 128 * dm:128 * (dm + 1)],
                                rhs=attnT[:, f, QC * c2 + a:QC * c2 + b],
                                start=(f == 0),
                                stop=(f == 1),
                            )
                    ob = opool.tile([128, QC], f16, tag="ob")
                    if tail_idx % 2 == 0:  # exp queue empty: use ScalarE
                        nc.scalar.copy(ob, po)
                    else:
                        nc.vector.tensor_copy(ob, po)
                    nc.sync.dma_start(
                        outT_d.ap()[128 * dm:128 * (dm + 1),
                                    QC * c2:QC * (c2 + 1)],
                        ob,
                    )

                fill = deque()

                def pump(n=1):
                    for _ in range(n):
                        if fill:
                            fill.popleft()()

                def make_norm(mi, c, avs, tail=False):
                    q0 = QC * c
                    rden = rdens[norm_ctr[0] % 2]
                    norm_ctr[0] += 1

                    def emit_recips():
                        with nc.allow_low_precision(
                                reason="fp16 recip feeds fp16 bcast matmul"):
                            for a, b in ((0, 512), (512, QC)):
                                nc.vector.reciprocal(rden[64:65, a:b],
                                                     avs[0][64:65, a:b])
                                nc.vector.reciprocal(rden[0:1, a:b],
                                                     avs[1][0:1, a:b])

                    def emit_rest():
                        bc = psmm.tile([128, QC], f32, tag="mm", name="bc")
                        bs = bcpool.tile([128, QC], f16, tag="bc")
                        for a, b in ((0, 512), (512, QC)):
                            nc.tensor.matmul(bc[:, a:b], lhsT=ones65,
                                             rhs=rden[:, a:b],
                                             start=True, stop=True)
                            if tail:
                                nc.scalar.copy(bs[:, a:b], bc[:, a:b])
                            else:
                                nc.vector.tensor_copy(bs[:, a:b], bc[:, a:b])
                            nc.vector.tensor_mul(
                                attnT[0:DK, mi, q0 + a:q0 + b],
                                avs[0][0:DK, a:b], bs[0:DK, a:b])
                            nc.vector.tensor_mul(
                                attnT[DK:128, mi, q0 + a:q0 + b],
                                avs[1][DK:128, a:b], bs[DK:128, a:b])

                    return emit_recips, emit_rest

                def attention(mi, c, prev_rest, pumps):
                    q0 = QC * c
                    njt = (q0 + QC) // KT
                    avs = [psav.tile([128, QC], f32, tag="av",
                                     name=f"av{hh}") for hh in range(2)]
                    jA_last = q0 // KT + 3  # last j with vs < 512

                    def emit_qk(j):
                        k0 = KT * j
                        vs = max(0, k0 - q0)
                        if vs >= 512:
                            # both heads packed into one psum tile at
                            # column offsets 0 / 512; one strided exp
                            w = QC - vs
                            ps = psmm.tile([128, QC], f32, tag="mm",
                                           name="ps0")
                            for hh in range(2):
                                pb = 64 * hh
                                o = 512 * hh
                                nc.tensor.matmul(
                                    ps[:, o:o + w],
                                    lhsT=kT_sb[pb:pb + DK, mi, k0:k0 + KT],
                                    rhs=qT_sb[pb:pb + DK, mi,
                                              q0 + vs:q0 + QC],
                                    start=True,
                                    stop=False,
                                )
                                nc.tensor.matmul(
                                    ps[:, o:o + KT],
                                    lhsT=stA,
                                    rhs=stB,
                                    start=False,
                                    stop=True,
                                )
                            e = epool.tile([128, QC], f16, tag="e")
                            pv = ps.rearrange("p (g z) -> p g z", z=512)
                            ev = e.rearrange("p (g z) -> p g z", z=512)
                            nc.scalar.activation(
                                ev[:, :, 0:w], pv[:, :, 0:w], Exp,
                                scale=0.125 * SCALE)
                            # e column offset per head relative to av cols
                            return vs, [e, e], [-vs, 512 - vs]
                        pss, es = [], []
                        for hh in range(2):
                            pb = 64 * hh
                            ps = psmm.tile([128, QC], f32, tag="mm",
                                           name=f"ps{hh}")
                            for a, b in seg2(vs):
                                diag_here = (k0 >= q0) and (a == vs)
                                nc.tensor.matmul(
                                    ps[:, a:b],
                                    lhsT=kT_sb[pb:pb + DK, mi, k0:k0 + KT],
                                    rhs=qT_sb[pb:pb + DK, mi,
                                              q0 + a:q0 + b],
                                    start=True,
                                    stop=not diag_here,
                                )
                                if diag_here:  # staircase causal mask
                                    nc.tensor.matmul(
                                        ps[:, vs:vs + KT],
                                        lhsT=stA,
                                        rhs=stB,
                                        start=False,
                                        stop=True,
                                    )
                            pss.append(ps)
                        for hh in range(2):
                            e = epool.tile([128, QC], f16, tag="e")
                            nc.scalar.activation(
                                e[:, vs:QC], pss[hh][:, vs:QC], Exp,
                                scale=0.125 * SCALE)
                            es.append(e)
                        return vs, es, [0, 0]

                    def emit_av(j, vs, es, deltas):
                        av_ranges = []
                        if vs < 512:
                            av_ranges.append((vs, 512, j == jA_last))
                        av_ranges.append((max(vs, 512), QC, j == njt - 1))
                        for hh in range(2):
                            blk = mi + 2 * hh
                            dlt = deltas[hh]
                            for a, b, fin in av_ranges:
                                nc.tensor.matmul(
                                    avs[hh][:, a:b],
                                    lhsT=v_sb[:, j,
                                              128 * blk:128 * (blk + 1)],
                                    rhs=es[hh][:, a + dlt:b + dlt],
                                    start=(j == 0),
                                    stop=fin,
                                )

                    pend = []
                    for j in range(njt):
                        new = emit_qk(j)
                        if j == 0 and prev_rest is not None:
                            prev_rest()
                        pump(pumps[j] if j < len(pumps) else 0)
                        if len(pend) == 2:  # AV trails QK/exp by 2 k-tiles
                            emit_av(*pend.pop(0))
                        pend.append((j,) + new)
                    while pend:
                        pump(1)
                        emit_av(*pend.pop(0))
                    return avs

                # ---- emission schedule ----
                proj_qk_dst(0, 0, 0, split_evac=True)  # q pair0 cols 0:1024
                proj_qk_dst(1, 0, 0, split_evac=True)  # k pair0
                fill.append(lambda: proj_qk_dst(0, 1, 0))
                fill.append(lambda: proj_qk_dst(1, 1, 0))
                fill.extend([lambda st=st: proj_v(st) for st in range(8)])
                avs = attention(0, 0, None, pumps=[1, 2] + [1] * 6)
                recips, nrm = make_norm(0, 0, avs)
                recips()
                fill.append(lambda: proj_qk_dst(0, 0, 1))
                fill.append(lambda: proj_qk_dst(1, 0, 1))
                fill.extend([lambda st=st: proj_v(st) for st in range(8, 14)])
                avs = attention(1, 0, nrm, pumps=[0] + [1] * 7)
                recips, nrm = make_norm(1, 0, avs)
                recips()
                fill.append(lambda: proj_qk_dst(0, 1, 1))
                fill.append(lambda: proj_qk_dst(1, 1, 1))
                fill.extend([lambda st=st: proj_v(st)
                             for st in range(14, 16)])
                fill.extend([lambda dm=dm: wo_block(dm, 0)
                             for dm in range(5)])
                avs = attention(0, 1, nrm,
                                pumps=[0] + [1, 0] * 7 + [1])
                recips, nrm = make_norm(0, 1, avs)
                recips()
                fill.extend([lambda dm=dm: wo_block(dm, 0)
                             for dm in range(5, 8)])
                avs = attention(1, 1, nrm,
                                pumps=[0] + [1, 0] * 7 + [1])
                pump(16)
                recips, nrm = make_norm(1, 1, avs, tail=True)
                recips()
                nrm()
                for dm in range(8):
                    wo_block(dm, 1, tail_idx=dm)

    nc.compile()
    return nc


def _get_nc():
    if "nc" not in _CACHE:
        _CACHE["nc"] = _build_nc()
    return _CACHE["nc"]


def _stairs():
    t = np.arange(128)
    stA = ((t[:, None] <= t[None, :]) * STA_V).astype(np.float16)
    stB = np.where(t[:, None] > t[None, :], STB_V, 0.0).astype(np.float16)
    return stA, stB


def _rearr_w(w):
    # [D, cols] -> [128, KC, cols]
    return np.ascontiguousarray(
        w.reshape(KC, 128, w.shape[1]).transpose(1, 0, 2))


def _make_in_maps(x, wq, wk, wv, wo):
    import ml_dtypes

    f8 = ml_dtypes.float8_e4m3
    f16 = np.float16
    stA, stB = _stairs()
    x = np.asarray(x, np.float32)
    wq = np.asarray(wq, np.float32)
    wk = np.asarray(wk, np.float32)
    wv = np.asarray(wv, np.float32)
    wo = np.asarray(wo, np.float32)

    xs, xls = [], []
    for b in range(B):
        x3 = np.ascontiguousarray(
            x[b].T.reshape(KC, 128, S).transpose(1, 0, 2)) * XS
        xh = x3.astype(f8)
        xl = (x3 - xh.astype(np.float32)).astype(f8)
        xs.append(xh)
        xls.append(xl)

    vperm = [0, 2, 1, 3]  # even heads first within the group
    in_maps = []
    for c in range(NCORES):
        b, g = divmod(c, HPC)
        cols = slice(g * GW, (g + 1) * GW)
        wvp = wv[:, cols].reshape(D, HPC, DK)[:, vperm, :].reshape(D, GW)
        wv3 = _rearr_w(wvp * WWS)
        wvh = wv3.astype(f8)
        wvl = (wv3 - wvh.astype(np.float32)).astype(f8)
        in_maps.append({
            "x8h": xs[b],
            "x8l": xls[b],
            "wq8": _rearr_w(wq[:, cols] * WWS).astype(f8),
            "wk8": _rearr_w(wk[:, cols] * WWS).astype(f8),
            "wv8h": wvh,
            "wv8l": wvl,
            "wo16": np.ascontiguousarray(
                wo[cols, :].reshape(2, 128, D).transpose(1, 0, 2)
            ).astype(f16),
            "stairA": stA,
            "stairB": stB,
        })
    return in_maps


def run(x, wq, wk, wv, wo, trace=False):
    from concourse.bass_utils import run_bass_kernel_spmd

    nc = _get_nc()
    in_maps = _make_in_maps(x, wq, wk, wv, wo)
    res = run_bass_kernel_spmd(nc, in_maps, list(range(NCORES)), trace=trace)
    acc = np.zeros((B, D, S), np.float64)
    for c in range(NCORES):
        acc[c // HPC] += res.results[c]["outT"].astype(np.float64)
    out = np.ascontiguousarray(acc.transpose(0, 2, 1).astype(np.float32))
    return out, res


def kernel(x, wq, wk, wv, wo):
    out, _ = run(x, wq, wk, wv, wo, trace=False)
    return out
